# revision 1
# baseline (speedup 1.0000x reference)
"""Trainium2 Bass kernel for nn_CharEmbedding (ragged_sequence).

Computation (see reference):
    rep = concat([emb[first], emb[mid].sum(1), emb[last]], -1)   # [U, 3H]
    out = rep @ head_w + head_b                                  # [U, O]
    tok = out[inv_i].reshape(B, L, O); pad time by (1,1)         # [B, L+2, O]

Strategy: fuse everything at token granularity, data-parallel over the
B*L = 32768 output tokens (4096 per core = exactly 2 sequences).  Host
precomputes per-token vocab indices (first/mid/last gathered through
inv_i) as int16 in the SWDGE dma_gather wrapped layout.  On each core:

  - emb table (bf16, padded to 4096 rows) resident in SBUF, swizzled for
    SBUF-source transposed dma_gather: partition = id % 128, rank = id // 128.
  - per 512-token tile: TWO dma_gathers of 7*512 = 3584 rows each
    (first+mids0-5 | mids6-11+last), transposed, producing [128, 2, 3584]
    bf16 feature-major (matmul-ready lhsT).  Splitting the gather lets
    Q7 descriptor-gen overlap SDMA drain (measured ~2x faster than one
    7168-row gather; multi-queue spreading raced on HW, so queue 0 only).
  - 12 mid embeddings summed pairwise (tree) on DVE in bf16 (2x mode).
  - PE: out[tok, :] = bias (K=1 matmul of ones x bias) + sum over 6
    K-chunks of embT.T @ W_chunk, accumulated in PSUM (fp32).
  - ACT evacuates PSUM -> SBUF fp32, HWDGE DMA stores to DRAM.

Output rows land contiguously; host assembles the [16, 2050, 768] padded
result (pad rows are zeros and never touch the device).
"""

import numpy as np
import ml_dtypes

import concourse.bacc as bacc
import concourse.mybir as mybir
import concourse.tile as tile
from concourse.bass_utils import run_bass_kernel_spmd

BF16 = ml_dtypes.bfloat16

# Problem constants (hardcoded per contract).
VOCAB = 4000
VOCAB_PAD = 4096
U = 30000
M = 12
H = 256
O = 768
B = 16
L = 2048
N_CORES = 8
T_CORE = (B * L) // N_CORES      # 4096 tokens per core
TILE_T = 512                     # tokens per pipeline tile
ROWS_PER_TOK = 2 + M             # 14 gathered rows per token
KCH = (3 * H) // 128             # 6 K-chunks of the 768-dim contraction
NQ = 1                           # SWDGE queues used for gathers

_NC_CACHE = {}


def build_nc(n_tiles=T_CORE // TILE_T, table_in_sbuf=True, nq=NQ, reps=1,
             gbufs=2, mbufs=2, obufs=4, pbufs=4):
    """Build (and compile) the per-core Bass module.

    Tokens handled = n_tiles * TILE_T.  All cores run the same program.
    reps > 1 wraps the pipeline in a For_i hardware loop (timing only).
    """
    t_core = n_tiles * TILE_T
    rows_tile = ROWS_PER_TOK * TILE_T          # 7168
    rows_half = rows_tile // 2                 # 3584 (= 7 streams of 512)
    idx_cols = rows_tile // 16                 # 448 idx columns per tile
    half_cols = idx_cols // 2

    nc = bacc.Bacc("TRN2", target_bir_lowering=False, debug=False,
                   num_swdge_queues=nq)

    if table_in_sbuf:
        tbl_d = nc.dram_tensor("tbl", [128, (VOCAB_PAD // 128) * H],
                               mybir.dt.bfloat16, kind="ExternalInput")
    else:
        tbl_d = nc.dram_tensor("tbl", [VOCAB_PAD, H], mybir.dt.bfloat16,
                               kind="ExternalInput")
    wts_d = nc.dram_tensor("wts", [128, KCH * O], mybir.dt.bfloat16,
                           kind="ExternalInput")
    bias_d = nc.dram_tensor("bias", [1, O], mybir.dt.bfloat16,
                            kind="ExternalInput")
    idx_d = nc.dram_tensor("idx", [128, n_tiles * idx_cols], mybir.dt.int16,
                           kind="ExternalInput")
    out_d = nc.dram_tensor("out", [t_core, O], mybir.dt.float32,
                           kind="ExternalOutput")

    with tile.TileContext(nc) as tc:
        with (
            tc.tile_pool(name="const", bufs=1) as cpool,
            tc.tile_pool(name="gath", bufs=gbufs) as gpool,
            tc.tile_pool(name="mids", bufs=mbufs) as mpool,
            tc.tile_pool(name="outs", bufs=obufs) as opool,
            tc.tile_pool(name="psum", bufs=pbufs, space="PSUM") as ppool,
        ):
            # ---- resident constants ----
            if table_in_sbuf:
                tbl = cpool.tile([128, (VOCAB_PAD // 128) * H], mybir.dt.bfloat16)
                nc.sync.dma_start(out=tbl[:], in_=tbl_d[:])
            wts = cpool.tile([128, KCH, O], mybir.dt.bfloat16)
            nc.sync.dma_start(out=wts[:], in_=wts_d[:].rearrange(
                "p (c o) -> p c o", c=KCH))
            bias_t = cpool.tile([1, O], mybir.dt.bfloat16)
            nc.sync.dma_start(out=bias_t[:], in_=bias_d[:])
            idx_t = cpool.tile([128, n_tiles * idx_cols], mybir.dt.int16)
            nc.sync.dma_start(out=idx_t[:], in_=idx_d[:])
            ones_t = cpool.tile([1, 128], mybir.dt.bfloat16)
            nc.vector.memset(ones_t[:], 1.0)

            import contextlib
            rep_ctx = tc.For_i(0, reps, 1) if reps > 1 else contextlib.nullcontext()
            with rep_ctx:
             for t in range(n_tiles):
                # ---- gather: 2 x 3584 rows, transposed (feature-major) ----
                # stream order per tile: first, mid0..mid11, last (512 each)
                # g0 = first + mid0..5, g1 = mid6..11 + last
                gs = []
                for h in range(2):
                    gh = gpool.tile([128, 2, rows_half], mybir.dt.bfloat16,
                                    name=f"g{h}", tag=f"g{h}")
                    col0 = t * idx_cols + h * half_cols
                    q = (2 * t + h) % nq
                    if table_in_sbuf:
                        nc.gpsimd.dma_gather(
                            gh[:], tbl[:],
                            idx_t[:, col0:col0 + half_cols],
                            rows_half, rows_half, H,
                            transpose=True,
                            sbuf_tokens_per_rank=128,
                            sbuf_free_dim_per_rank=2 * H,
                            single_packet=False, queue_num=q)
                    else:
                        nc.gpsimd.dma_gather(
                            gh[:], tbl_d[:],
                            idx_t[:, col0:col0 + half_cols],
                            rows_half, rows_half, H,
                            transpose=True,
                            single_packet=False, queue_num=q)
                    gs.append(gh)

                first = gs[0][:, :, 0:TILE_T]
                last = gs[1][:, :, 6 * TILE_T:7 * TILE_T]

                def mid(j):
                    gh = gs[j // 6]
                    off = (1 + j) * TILE_T if j < 6 else (j - 6) * TILE_T
                    return gh[:, :, off:off + TILE_T]

                # ---- mid-sum: pairwise tree on DVE (bf16, 2x mode) ----
                msA = mpool.tile([128, 2, 6 * TILE_T], mybir.dt.bfloat16)
                for k in range(6):
                    nc.vector.tensor_add(
                        msA[:, :, k * TILE_T:(k + 1) * TILE_T],
                        mid(2 * k), mid(2 * k + 1))
                msB = mpool.tile([128, 2, 3 * TILE_T], mybir.dt.bfloat16)
                for k in range(3):
                    nc.vector.tensor_add(
                        msB[:, :, k * TILE_T:(k + 1) * TILE_T],
                        msA[:, :, 2 * k * TILE_T:(2 * k + 1) * TILE_T],
                        msA[:, :, (2 * k + 1) * TILE_T:(2 * k + 2) * TILE_T])
                msum = mpool.tile([128, 2, TILE_T], mybir.dt.bfloat16)
                nc.vector.tensor_add(
                    msum[:], msB[:, :, 0:TILE_T], msB[:, :, TILE_T:2 * TILE_T])
                nc.vector.tensor_add(
                    msum[:], msum[:], msB[:, :, 2 * TILE_T:3 * TILE_T])

                groups = (first, msum[:], last)

                # ---- matmuls + evacuate + store, per 128-token subtile ----
                for m in range(TILE_T // 128):
                    tok = slice(m * 128, (m + 1) * 128)
                    ps_a = ppool.tile([128, 512], mybir.dt.float32)
                    ps_b = ppool.tile([128, 256], mybir.dt.float32)
                    for ps, osl in ((ps_a, slice(0, 512)), (ps_b, slice(512, O))):
                        nc.tensor.matmul(ps[:], ones_t[:], bias_t[:, osl],
                                         start=True, stop=False)
                        for c in range(KCH):
                            gsrc = groups[c // 2]
                            nc.tensor.matmul(
                                ps[:], gsrc[:, c % 2, tok], wts[:, c, osl],
                                start=False, stop=(c == KCH - 1))
                    o_sb = opool.tile([128, O], mybir.dt.float32)
                    nc.scalar.copy(o_sb[:, 0:512], ps_a[:])
                    nc.scalar.copy(o_sb[:, 512:O], ps_b[:])
                    row = t * TILE_T + m * 128
                    nc.sync.dma_start(out=out_d[row:row + 128, :], in_=o_sb[:])

    nc.compile()
    return nc


def _get_nc(n_tiles=T_CORE // TILE_T, table_in_sbuf=True):
    key = (n_tiles, table_in_sbuf)
    if key not in _NC_CACHE:
        _NC_CACHE[key] = build_nc(*key)
    return _NC_CACHE[key]


def _wrap_idx(stream):
    """Pack an index stream into the SWDGE gather layout: idx i lives at
    [i % 16, i // 16], replicated across the 8 groups of 16 partitions."""
    n = stream.shape[0]
    arr = stream.reshape(n // 16, 16).T.astype(np.int16)   # [16, n//16]
    return np.tile(arr, (8, 1))                            # [128, n//16]


def prep_inputs(emb_table, head_w, head_b, first, mid, last, inv_i,
                n_tiles=T_CORE // TILE_T, table_in_sbuf=True):
    """Host-side shard + layout prep.  Returns in_maps for 8 cores."""
    emb = np.asarray(emb_table, dtype=np.float32).copy()
    emb[0] = 0.0  # padding_idx (reference masks id 0; row 0 is zero anyway)
    tbl16 = np.zeros((VOCAB_PAD, H), dtype=BF16)
    tbl16[:VOCAB] = emb.astype(BF16)
    if table_in_sbuf:
        # partition = id % 128, rank (free-dim block) = id // 128
        tbl_in = np.ascontiguousarray(
            tbl16.reshape(VOCAB_PAD // 128, 128, H).transpose(1, 0, 2)
        ).reshape(128, (VOCAB_PAD // 128) * H)
    else:
        tbl_in = tbl16

    Wb = np.asarray(head_w, dtype=np.float32).astype(BF16)      # [768, 768]
    wts_in = np.ascontiguousarray(
        Wb.reshape(KCH, 128, O).transpose(1, 0, 2)).reshape(128, KCH * O)
    bias_in = np.asarray(head_b, dtype=np.float32).astype(BF16).reshape(1, O)

    inv_i = np.asarray(inv_i)
    fi = np.asarray(first)[inv_i].astype(np.int16)   # [B*L]
    mi = np.asarray(mid)[inv_i].astype(np.int16)     # [B*L, 12]
    la = np.asarray(last)[inv_i].astype(np.int16)    # [B*L]

    in_maps = []
    for c in range(N_CORES):
        base = c * T_CORE
        cols = []
        for t in range(n_tiles):
            s = slice(base + t * TILE_T, base + (t + 1) * TILE_T)
            stream = np.concatenate(
                [fi[s]] + [mi[s, j] for j in range(M)] + [la[s]])
            cols.append(_wrap_idx(stream))
        idx_in = np.concatenate(cols, axis=1)
        in_maps.append({
            "tbl": tbl_in, "wts": wts_in, "bias": bias_in, "idx": idx_in,
        })
    return in_maps


def kernel(emb_table, head_w, head_b, first, mid, last, inv_i,
           batch, seq_len, _nc=None, _return_raw=False):
    batch = int(batch)
    seq_len = int(seq_len)
    assert batch == B and seq_len == L, (batch, seq_len)
    nc = _nc if _nc is not None else _get_nc()
    in_maps = prep_inputs(emb_table, head_w, head_b, first, mid, last, inv_i)
    res = run_bass_kernel_spmd(nc, in_maps, core_ids=list(range(N_CORES)))
    per_core = [r["out"] for r in res.results]         # each [4096, 768] f32
    if _return_raw:
        return per_core
    full = np.zeros((B, L + 2, O), dtype=np.float32)
    seq_per_core = T_CORE // L                         # 2 sequences per core
    for c in range(N_CORES):
        full[c * seq_per_core:(c + 1) * seq_per_core, 1:L + 1, :] = (
            per_core[c].reshape(seq_per_core, L, O))
    return full



# revision 3
# speedup vs baseline: 1.7709x; 1.7709x over previous
"""Trainium2 Bass kernel for nn_CharEmbedding (ragged_sequence).

Computation (see reference):
    rep = concat([emb[first], emb[mid].sum(1), emb[last]], -1)   # [U, 3H]
    out = rep @ head_w + head_b                                  # [U, O]
    tok = out[inv_i].reshape(B, L, O); pad time by (1,1)         # [B, L+2, O]

Strategy: data-parallel over the B*L = 32768 output tokens (4096 per
core = exactly 2 sequences), fused at token granularity.

The 12 mid-char embeddings per token dominate SWDGE gather descriptor
generation (the Q7 desc-gen rate of ~4-8 ns/row is the kernel's
bottleneck when all 14 streams are gathered).  They are instead
computed on the Tensor engine as a counts-matmul:

    msum[feat, tok] = sum_v E[v, feat] * counts[v, tok]

with host-precomputed per-tile count matrices (fp8 rhs - integers <= 16
are exact in e4m3; lhsT is the bf16 table, vocab-major).  The PE output
is natively feature-major, i.e. matmul-ready lhsT for the head matmul.
first/last (2 of 14 streams) stay on dma_gather (bf16, SBUF-source
transposed).  Bias is added on DVE during PSUM evacuation (frees PE of
the K=1 bias matmuls).

Per 512-token tile:
  - dma_gather of 1024 rows (first|last), transposed -> [128, 2, 1024]
  - counts DMA [128, 32, 512] fp8 + 2x32 matmuls -> msum psum, ACT
    evacuates to bf16 [128, 2, 512]
  - head: per 128-token subtile, 6 K-chunk matmuls into 2 psum banks,
    DVE adds bias while evacuating to SBUF f32, HWDGE store to DRAM.

Output rows land contiguously; host assembles the [16, 2050, 768]
padded result (pad rows are zeros and never touch the device).
"""

import numpy as np
import ml_dtypes

import concourse.bacc as bacc
import concourse.mybir as mybir
import concourse.tile as tile
from concourse.bass_utils import run_bass_kernel_spmd

BF16 = ml_dtypes.bfloat16
F8 = ml_dtypes.float8_e4m3

# Problem constants (hardcoded per contract).
VOCAB = 4000
VOCAB_PAD = 4096
U = 30000
M = 12
H = 256
O = 768
B = 16
L = 2048
N_CORES = 8
T_CORE = (B * L) // N_CORES      # 4096 tokens per core
TILE_T = 512                     # tokens per pipeline tile
KCH = (3 * H) // 128             # 6 K-chunks of the 768-dim contraction
VCH = VOCAB_PAD // 128           # 32 vocab chunks for the counts matmul
GROWS = 2 * TILE_T               # gathered rows per tile (first|last)
IDXC = GROWS // 16               # idx columns per tile

_NC_CACHE = {}

# fp8e4m3 encodings of integers 0..15 (counts never exceed 12)
_F8_LUT = np.arange(16).astype(F8).view(np.uint8)


def build_nc(n_tiles=T_CORE // TILE_T, reps=1, cnt_bf16=False):
    """Build (and compile) the per-core Bass module.

    Tokens handled = n_tiles * TILE_T.  All cores run the same program.
    reps > 1 wraps the pipeline in a For_i hardware loop (timing only).
    cnt_bf16 switches the counts rhs to bf16 (fallback if the mixed
    bf16 x fp8 matmul misbehaves).
    """
    import contextlib
    t_core = n_tiles * TILE_T
    cdt = mybir.dt.bfloat16 if cnt_bf16 else mybir.dt.float8e4

    nc = bacc.Bacc("TRN2", target_bir_lowering=False, debug=False)

    tbl_d = nc.dram_tensor("tbl", [128, VCH * H], mybir.dt.bfloat16,
                           kind="ExternalInput")
    ecnt_d = nc.dram_tensor("ecnt", [128, VCH * H], mybir.dt.bfloat16,
                            kind="ExternalInput")
    wts_d = nc.dram_tensor("wts", [128, KCH * O], mybir.dt.bfloat16,
                           kind="ExternalInput")
    bias_d = nc.dram_tensor("bias", [128, O], mybir.dt.float32,
                            kind="ExternalInput")
    idx_d = nc.dram_tensor("idx", [128, n_tiles * IDXC], mybir.dt.int16,
                           kind="ExternalInput")
    cnt_d = nc.dram_tensor("cnt", [128, n_tiles * VCH * TILE_T], cdt,
                           kind="ExternalInput")
    out_d = nc.dram_tensor("out", [t_core, O], mybir.dt.float32,
                           kind="ExternalOutput")

    with tile.TileContext(nc) as tc:
        with (
            tc.tile_pool(name="const", bufs=1) as cpool,
            tc.tile_pool(name="gath", bufs=2) as gpool,
            tc.tile_pool(name="cnts", bufs=2) as cntpool,
            tc.tile_pool(name="mids", bufs=2) as mpool,
            tc.tile_pool(name="outs", bufs=4) as opool,
            tc.tile_pool(name="psum_m", bufs=2, space="PSUM") as pmpool,
            tc.tile_pool(name="psum_h", bufs=3, space="PSUM") as phpool,
        ):
            # ---- resident constants ----
            tbl = cpool.tile([128, VCH * H], mybir.dt.bfloat16)
            nc.sync.dma_start(out=tbl[:], in_=tbl_d[:])
            ecnt = cpool.tile([128, VCH, H], mybir.dt.bfloat16)
            nc.sync.dma_start(out=ecnt[:], in_=ecnt_d[:].rearrange(
                "p (c f) -> p c f", c=VCH))
            wts = cpool.tile([128, KCH, O], mybir.dt.bfloat16)
            nc.sync.dma_start(out=wts[:], in_=wts_d[:].rearrange(
                "p (c o) -> p c o", c=KCH))
            bias_t = cpool.tile([128, O], mybir.dt.float32)
            nc.sync.dma_start(out=bias_t[:], in_=bias_d[:])
            idx_t = cpool.tile([128, n_tiles * IDXC], mybir.dt.int16)
            nc.sync.dma_start(out=idx_t[:], in_=idx_d[:])

            rep_ctx = tc.For_i(0, reps, 1) if reps > 1 else contextlib.nullcontext()
            with rep_ctx:
             for t in range(n_tiles):
                # ---- gather first|last: 1024 rows, feature-major ----
                gh = gpool.tile([128, 2, GROWS], mybir.dt.bfloat16, tag="g")
                col0 = t * IDXC
                nc.gpsimd.dma_gather(
                    gh[:], tbl[:], idx_t[:, col0:col0 + IDXC],
                    GROWS, GROWS, H,
                    transpose=True,
                    sbuf_tokens_per_rank=128,
                    sbuf_free_dim_per_rank=2 * H,
                    single_packet=False, queue_num=0)

                # ---- counts tile DMA ----
                cnt = cntpool.tile([128, VCH, TILE_T], cdt, tag="c")
                s0 = t * VCH * TILE_T
                nc.sync.dma_start(
                    out=cnt[:],
                    in_=cnt_d[:, s0:s0 + VCH * TILE_T].rearrange(
                        "p (c n) -> p c n", c=VCH))

                # ---- mid-sum as counts matmul (feature-major psum) ----
                msum = mpool.tile([128, 2, TILE_T], mybir.dt.bfloat16)
                for f2 in range(2):
                    psm = pmpool.tile([128, TILE_T], mybir.dt.float32)
                    for c in range(VCH):
                        nc.tensor.matmul(
                            psm[:], ecnt[:, c, f2 * 128:(f2 + 1) * 128],
                            cnt[:, c, :],
                            start=(c == 0), stop=(c == VCH - 1))
                    nc.scalar.copy(msum[:, f2, :], psm[:])

                first = gh[:, :, 0:TILE_T]
                last = gh[:, :, TILE_T:2 * TILE_T]
                groups = (first, msum[:], last)

                # ---- head matmuls + bias-evacuate + store ----
                for m in range(TILE_T // 128):
                    tok = slice(m * 128, (m + 1) * 128)
                    ps_a = phpool.tile([128, 512], mybir.dt.float32)
                    ps_b = phpool.tile([128, 256], mybir.dt.float32)
                    for ps, osl in ((ps_a, slice(0, 512)), (ps_b, slice(512, O))):
                        for c in range(KCH):
                            gsrc = groups[c // 2]
                            nc.tensor.matmul(
                                ps[:], gsrc[:, c % 2, tok], wts[:, c, osl],
                                start=(c == 0), stop=(c == KCH - 1))
                    o_sb = opool.tile([128, O], mybir.dt.float32)
                    nc.vector.tensor_add(o_sb[:, 0:512], ps_a[:],
                                         bias_t[:, 0:512])
                    nc.vector.tensor_add(o_sb[:, 512:O], ps_b[:],
                                         bias_t[:, 512:O])
                    row = t * TILE_T + m * 128
                    nc.sync.dma_start(out=out_d[row:row + 128, :], in_=o_sb[:])

    nc.compile()
    return nc


def _get_nc(n_tiles=T_CORE // TILE_T, cnt_bf16=False):
    key = (n_tiles, cnt_bf16)
    if key not in _NC_CACHE:
        _NC_CACHE[key] = build_nc(n_tiles=n_tiles, cnt_bf16=cnt_bf16)
    return _NC_CACHE[key]


def _wrap_idx(stream):
    """Pack an index stream into the SWDGE gather layout: idx i lives at
    [i % 16, i // 16], replicated across the 8 groups of 16 partitions."""
    n = stream.shape[0]
    arr = stream.reshape(n // 16, 16).T.astype(np.int16)   # [16, n//16]
    return np.tile(arr, (8, 1))                            # [128, n//16]


def prep_inputs(emb_table, head_w, head_b, first, mid, last, inv_i,
                n_tiles=T_CORE // TILE_T, cnt_bf16=False):
    """Host-side shard + layout prep.  Returns in_maps for 8 cores."""
    emb = np.asarray(emb_table, dtype=np.float32).copy()
    emb[0] = 0.0  # padding_idx (reference masks id 0; row 0 is zero anyway)
    tbl16 = np.zeros((VOCAB_PAD, H), dtype=BF16)
    tbl16[:VOCAB] = emb.astype(BF16)
    # gather table: partition = id % 128, rank (free-dim block) = id // 128
    tbl_in = np.ascontiguousarray(
        tbl16.reshape(VCH, 128, H).transpose(1, 0, 2)
    ).reshape(128, VCH * H)
    # counts table: partition = id % 128 within chunk id // 128 (vocab-major)
    ecnt_in = np.ascontiguousarray(
        tbl16.reshape(VCH, 128, H).transpose(1, 0, 2)
    ).reshape(128, VCH * H)

    Wb = np.asarray(head_w, dtype=np.float32).astype(BF16)      # [768, 768]
    wts_in = np.ascontiguousarray(
        Wb.reshape(KCH, 128, O).transpose(1, 0, 2)).reshape(128, KCH * O)
    bias_in = np.tile(np.asarray(head_b, dtype=np.float32).reshape(1, O),
                      (128, 1))

    inv_i = np.asarray(inv_i)
    fi = np.asarray(first)[inv_i].astype(np.int16)   # [B*L]
    mi = np.asarray(mid)[inv_i].astype(np.int16)     # [B*L, 12]
    la = np.asarray(last)[inv_i].astype(np.int16)    # [B*L]

    t_core = n_tiles * TILE_T
    cdt = BF16 if cnt_bf16 else F8
    tok_of_slot = np.repeat(np.arange(TILE_T), M)
    in_maps = []
    for c in range(N_CORES):
        base = c * T_CORE
        cols = []
        cnts = []
        for t in range(n_tiles):
            s = slice(base + t * TILE_T, base + (t + 1) * TILE_T)
            cols.append(_wrap_idx(np.concatenate([fi[s], la[s]])))
            cm = np.zeros((VOCAB_PAD, TILE_T), dtype=np.uint8)
            np.add.at(cm, (mi[s].ravel(), tok_of_slot), 1)
            # [4096, 512] -> [128 part, 32 chunk, 512]
            cm = np.ascontiguousarray(
                cm.reshape(VCH, 128, TILE_T).transpose(1, 0, 2))
            if cnt_bf16:
                cnts.append(cm.astype(BF16))
            else:
                cnts.append(_F8_LUT[cm].view(F8))
        idx_in = np.concatenate(cols, axis=1)
        cnt_in = np.concatenate(
            [x.reshape(128, VCH * TILE_T) for x in cnts], axis=1)
        in_maps.append({
            "tbl": tbl_in, "ecnt": ecnt_in, "wts": wts_in, "bias": bias_in,
            "idx": idx_in, "cnt": np.ascontiguousarray(cnt_in.view(cdt)),
        })
    return in_maps


def kernel(emb_table, head_w, head_b, first, mid, last, inv_i,
           batch, seq_len, _nc=None, _return_raw=False):
    batch = int(batch)
    seq_len = int(seq_len)
    assert batch == B and seq_len == L, (batch, seq_len)
    nc = _nc if _nc is not None else _get_nc()
    in_maps = prep_inputs(emb_table, head_w, head_b, first, mid, last, inv_i)
    res = run_bass_kernel_spmd(nc, in_maps, core_ids=list(range(N_CORES)))
    per_core = [r["out"] for r in res.results]         # each [4096, 768] f32
    if _return_raw:
        return per_core
    full = np.zeros((B, L + 2, O), dtype=np.float32)
    seq_per_core = T_CORE // L                         # 2 sequences per core
    for c in range(N_CORES):
        full[c * seq_per_core:(c + 1) * seq_per_core, 1:L + 1, :] = (
            per_core[c].reshape(seq_per_core, L, O))
    return full


# revision 4
# speedup vs baseline: 3.3992x; 1.9195x over previous
"""Trainium2 Bass kernel for nn_CharEmbedding (ragged_sequence).

Computation (see reference):
    rep = concat([emb[first], emb[mid].sum(1), emb[last]], -1)   # [U, 3H]
    out = rep @ head_w + head_b                                  # [U, O]
    tok = out[inv_i].reshape(B, L, O); pad time by (1,1)         # [B, L+2, O]

Strategy: data-parallel over the B*L = 32768 output tokens (4096 per
core = exactly 2 sequences), fused at token granularity.  Two
structural moves keep both the SWDGE descriptor-generation rate
(~4-8 ns/row on the Q7, the original bottleneck) and the Tensor engine
off the critical path:

1. The 12 mid-char embeddings per token (86% of gather rows) become a
   counts-matmul on PE:  msum[feat, tok] = sum_v E[v, feat] * cnt[v, tok]
   with host-precomputed per-tile count matrices (fp8 rhs - integer
   counts <= 12 are exact in e4m3; bf16 lhsT).  The PE output is
   natively feature-major, i.e. matmul-ready for the head contraction.

2. The head weight blocks for first/last are folded on the host into
   lookup tables  T0 = E @ W[:256] + b  and  T2 = E @ W[512:]  (weight
   refactoring; data-independent).  first/last then bypass the head
   matmul entirely: one plain dma_gather per tile fetches their
   768-wide output-space rows token-major, and the DVE adds them onto
   the head PSUM during evacuation.  Head matmul shrinks to the 2
   mid-block K-chunks.

Per 512-token tile:
  - plain dma_gather of 1024 rows (first|last) from the [8192, 768]
    T0|T2 table in DRAM -> [128, 8, 768] bf16 token-major
  - counts DMA [128, 32, 512] fp8 + 2x32 matmuls -> msum psum, ACT
    evacuates to bf16 [128, 2, 512] (feature-major)
  - head: per 128-token subtile, 2 K-chunk matmuls into 2 psum banks;
    DVE evacuates with +T0row +T2row; HWDGE stores f32 rows to DRAM.

Output rows land contiguously; host assembles the [16, 2050, 768]
padded result (pad rows are zeros and never touch the device).
"""

import numpy as np
import ml_dtypes

import concourse.bacc as bacc
import concourse.mybir as mybir
import concourse.tile as tile
from concourse.bass_utils import run_bass_kernel_spmd

BF16 = ml_dtypes.bfloat16
F8 = ml_dtypes.float8_e4m3

# Problem constants (hardcoded per contract).
VOCAB = 4000
VOCAB_PAD = 4096
U = 30000
M = 12
H = 256
O = 768
B = 16
L = 2048
N_CORES = 8
T_CORE = (B * L) // N_CORES      # 4096 tokens per core
TILE_T = 512                     # tokens per pipeline tile
KMID = 2                         # head K-chunks (mid block only)
VCH = VOCAB_PAD // 128           # 32 vocab chunks for the counts matmul
GROWS = 2 * TILE_T               # gathered rows per tile (first|last)
IDXC = GROWS // 16               # idx columns per tile
GR = GROWS // 128                # gather output ranks (8)

_NC_CACHE = {}

# fp8e4m3 encodings of integers 0..15 (counts never exceed 12)
_F8_LUT = np.arange(16).astype(F8).view(np.uint8)


def build_nc(n_tiles=T_CORE // TILE_T, reps=1, cnt_bf16=False):
    """Build (and compile) the per-core Bass module.

    Tokens handled = n_tiles * TILE_T.  All cores run the same program.
    reps > 1 wraps the pipeline in a For_i hardware loop (timing only).
    cnt_bf16 switches the counts rhs to bf16 (fallback if the mixed
    bf16 x fp8 matmul misbehaves).
    """
    import contextlib
    t_core = n_tiles * TILE_T
    cdt = mybir.dt.bfloat16 if cnt_bf16 else mybir.dt.float8e4

    nc = bacc.Bacc("TRN2", target_bir_lowering=False, debug=False)

    t02_d = nc.dram_tensor("t02", [2 * VOCAB_PAD, O], mybir.dt.bfloat16,
                           kind="ExternalInput")
    ecnt_d = nc.dram_tensor("ecnt", [128, VCH * H], mybir.dt.bfloat16,
                            kind="ExternalInput")
    wts_d = nc.dram_tensor("wts", [128, KMID * O], mybir.dt.bfloat16,
                           kind="ExternalInput")
    idx_d = nc.dram_tensor("idx", [128, n_tiles * IDXC], mybir.dt.int16,
                           kind="ExternalInput")
    cnt_d = nc.dram_tensor("cnt", [128, n_tiles * VCH * TILE_T], cdt,
                           kind="ExternalInput")
    out_d = nc.dram_tensor("out", [t_core, O], mybir.dt.float32,
                           kind="ExternalOutput")

    with tile.TileContext(nc) as tc:
        with (
            tc.tile_pool(name="const", bufs=1) as cpool,
            tc.tile_pool(name="gath", bufs=2) as gpool,
            tc.tile_pool(name="cnts", bufs=2) as cntpool,
            tc.tile_pool(name="mids", bufs=2) as mpool,
            tc.tile_pool(name="outs", bufs=4) as opool,
            tc.tile_pool(name="psum_m", bufs=2, space="PSUM") as pmpool,
            tc.tile_pool(name="psum_h", bufs=3, space="PSUM") as phpool,
        ):
            # ---- resident constants ----
            ecnt = cpool.tile([128, VCH, H], mybir.dt.bfloat16)
            nc.sync.dma_start(out=ecnt[:], in_=ecnt_d[:].rearrange(
                "p (c f) -> p c f", c=VCH))
            wts = cpool.tile([128, KMID, O], mybir.dt.bfloat16)
            nc.sync.dma_start(out=wts[:], in_=wts_d[:].rearrange(
                "p (c o) -> p c o", c=KMID))
            idx_t = cpool.tile([128, n_tiles * IDXC], mybir.dt.int16)
            nc.sync.dma_start(out=idx_t[:], in_=idx_d[:])

            rep_ctx = tc.For_i(0, reps, 1) if reps > 1 else contextlib.nullcontext()
            with rep_ctx:
             for t in range(n_tiles):
                # ---- plain gather first|last rows of T0|T2, token-major ----
                gfl = gpool.tile([128, GR, O], mybir.dt.bfloat16, tag="g")
                col0 = t * IDXC
                nc.gpsimd.dma_gather(
                    gfl[:], t02_d[:], idx_t[:, col0:col0 + IDXC],
                    GROWS, GROWS, O,
                    transpose=False,
                    single_packet=False, queue_num=0)

                # ---- counts tile DMA ----
                cnt = cntpool.tile([128, VCH, TILE_T], cdt, tag="c")
                s0 = t * VCH * TILE_T
                nc.sync.dma_start(
                    out=cnt[:],
                    in_=cnt_d[:, s0:s0 + VCH * TILE_T].rearrange(
                        "p (c n) -> p c n", c=VCH))

                # ---- mid-sum as counts matmul (feature-major psum) ----
                msum = mpool.tile([128, 2, TILE_T], mybir.dt.bfloat16)
                for f2 in range(2):
                    psm = pmpool.tile([128, TILE_T], mybir.dt.float32)
                    for c in range(VCH):
                        nc.tensor.matmul(
                            psm[:], ecnt[:, c, f2 * 128:(f2 + 1) * 128],
                            cnt[:, c, :],
                            start=(c == 0), stop=(c == VCH - 1))
                    nc.scalar.copy(msum[:, f2, :], psm[:])

                # ---- mid head matmuls + gather-row evacuate + store ----
                for m in range(TILE_T // 128):
                    tok = slice(m * 128, (m + 1) * 128)
                    ps_a = phpool.tile([128, 512], mybir.dt.float32)
                    ps_b = phpool.tile([128, 256], mybir.dt.float32)
                    for ps, osl in ((ps_a, slice(0, 512)), (ps_b, slice(512, O))):
                        for c in range(KMID):
                            nc.tensor.matmul(
                                ps[:], msum[:, c, tok], wts[:, c, osl],
                                start=(c == 0), stop=(c == KMID - 1))
                    o_sb = opool.tile([128, O], mybir.dt.float32)
                    # o = head_mid + T0[first] (+bias, folded) + T2[last]
                    nc.vector.tensor_add(o_sb[:, 0:512], ps_a[:],
                                         gfl[:, m, 0:512])
                    nc.vector.tensor_add(o_sb[:, 512:O], ps_b[:],
                                         gfl[:, m, 512:O])
                    nc.vector.tensor_add(o_sb[:], o_sb[:], gfl[:, 4 + m, :])
                    row = t * TILE_T + m * 128
                    nc.sync.dma_start(out=out_d[row:row + 128, :], in_=o_sb[:])

    nc.compile()
    return nc


def _get_nc(n_tiles=T_CORE // TILE_T, cnt_bf16=False):
    key = (n_tiles, cnt_bf16)
    if key not in _NC_CACHE:
        _NC_CACHE[key] = build_nc(n_tiles=n_tiles, cnt_bf16=cnt_bf16)
    return _NC_CACHE[key]


def _wrap_idx(stream):
    """Pack an index stream into the SWDGE gather layout: idx i lives at
    [i % 16, i // 16], replicated across the 8 groups of 16 partitions."""
    n = stream.shape[0]
    arr = stream.reshape(n // 16, 16).T.astype(np.int16)   # [16, n//16]
    return np.tile(arr, (8, 1))                            # [128, n//16]


def prep_inputs(emb_table, head_w, head_b, first, mid, last, inv_i,
                n_tiles=T_CORE // TILE_T, cnt_bf16=False):
    """Host-side shard + layout prep.  Returns in_maps for 8 cores."""
    emb = np.asarray(emb_table, dtype=np.float32).copy()
    emb[0] = 0.0  # padding_idx (reference masks id 0; row 0 is zero anyway)
    W = np.asarray(head_w, dtype=np.float32)                 # [768, 768]
    b = np.asarray(head_b, dtype=np.float32)                 # [768]

    # Weight refactoring: fold the first/last head blocks (and the bias)
    # into output-space lookup tables.  T0[0] = b keeps padding_idx
    # semantics exact (E[0] = 0, bias still applies).
    t02 = np.zeros((2 * VOCAB_PAD, O), dtype=BF16)
    t02[:VOCAB] = (emb @ W[0:H] + b).astype(BF16)
    t02[VOCAB:VOCAB_PAD] = b.astype(BF16)                    # never indexed
    t02[VOCAB_PAD:VOCAB_PAD + VOCAB] = (emb @ W[2 * H:3 * H]).astype(BF16)

    # counts table: vocab-major, partition = id % 128, chunk = id // 128
    tbl16 = np.zeros((VOCAB_PAD, H), dtype=BF16)
    tbl16[:VOCAB] = emb.astype(BF16)
    ecnt_in = np.ascontiguousarray(
        tbl16.reshape(VCH, 128, H).transpose(1, 0, 2)
    ).reshape(128, VCH * H)

    Wmid = W[H:2 * H].astype(BF16)                           # [256, 768]
    wts_in = np.ascontiguousarray(
        Wmid.reshape(KMID, 128, O).transpose(1, 0, 2)).reshape(128, KMID * O)

    inv_i = np.asarray(inv_i)
    fi = np.asarray(first)[inv_i].astype(np.int16)   # [B*L]
    mi = np.asarray(mid)[inv_i].astype(np.int16)     # [B*L, 12]
    la = (np.asarray(last)[inv_i] + VOCAB_PAD).astype(np.int16)

    cdt = BF16 if cnt_bf16 else F8
    tok_of_slot = np.repeat(np.arange(TILE_T), M)
    in_maps = []
    for c in range(N_CORES):
        base = c * T_CORE
        cols = []
        cnts = []
        for t in range(n_tiles):
            s = slice(base + t * TILE_T, base + (t + 1) * TILE_T)
            cols.append(_wrap_idx(np.concatenate([fi[s], la[s]])))
            cm = np.zeros((VOCAB_PAD, TILE_T), dtype=np.uint8)
            np.add.at(cm, (mi[s].ravel(), tok_of_slot), 1)
            # [4096, 512] -> [128 part, 32 chunk, 512]
            cm = np.ascontiguousarray(
                cm.reshape(VCH, 128, TILE_T).transpose(1, 0, 2))
            if cnt_bf16:
                cnts.append(cm.astype(BF16))
            else:
                cnts.append(_F8_LUT[cm].view(F8))
        idx_in = np.concatenate(cols, axis=1)
        cnt_in = np.concatenate(
            [x.reshape(128, VCH * TILE_T) for x in cnts], axis=1)
        in_maps.append({
            "t02": t02, "ecnt": ecnt_in, "wts": wts_in,
            "idx": idx_in, "cnt": np.ascontiguousarray(cnt_in.view(cdt)),
        })
    return in_maps


def kernel(emb_table, head_w, head_b, first, mid, last, inv_i,
           batch, seq_len, _nc=None, _return_raw=False):
    batch = int(batch)
    seq_len = int(seq_len)
    assert batch == B and seq_len == L, (batch, seq_len)
    nc = _nc if _nc is not None else _get_nc()
    in_maps = prep_inputs(emb_table, head_w, head_b, first, mid, last, inv_i)
    res = run_bass_kernel_spmd(nc, in_maps, core_ids=list(range(N_CORES)))
    per_core = [r["out"] for r in res.results]         # each [4096, 768] f32
    if _return_raw:
        return per_core
    full = np.zeros((B, L + 2, O), dtype=np.float32)
    seq_per_core = T_CORE // L                         # 2 sequences per core
    for c in range(N_CORES):
        full[c * seq_per_core:(c + 1) * seq_per_core, 1:L + 1, :] = (
            per_core[c].reshape(seq_per_core, L, O))
    return full
